# revision 18
# baseline (speedup 1.0000x reference)
"""Causal multi-head attention on 8 trn2 NeuronCores.

Sharding: core = (batch b in {0,1}) x (head-group g in {0..3}; 4 heads each).
QKV weights column-sharded, Wo row-sharded (Megatron TP); each core emits a
partial output for its batch; the host sums the 4 partials per batch and adds
the output bias (the unshard step for row-parallel sharding).

Structure: one fused loop over sequence blocks of 512.  Iteration sc projects
q/k/v for block sc, then runs flash-style causal attention for q-block sc
(whose k-extent is exactly what has been projected so far), with the output
projection of block sc-1 wedged between to fill the PE while the previous
block's softmax tail drains.  Input x streams in 512-column slices in
consumption order on two DMA queues so the first projection starts ~8us in.

On-chip layout is feature-major: xT (E,S), qT/kT (256,S).  v is kept in
natural (s,f) layout as [v | ones*64] bf16 so the AV matmul uses the full
128-wide array and produces the softmax denominators replicated across 64
psum partitions - the normalize is then reciprocal + one elementwise mul.
Scores are computed transposed (k,q); the two heads of a pair go to row
groups 0/64 of the PE array back-to-back (concurrent on hardware) and share
one batched exp.  Probabilities are bf16.  Softmax skips max-subtraction
(scores are O(5); exp is safe in fp32).
"""

import numpy as np

B, S, E, H, D = 2, 2048, 1024, 16, 64
NCORES = 8
G = 4            # head-groups (cores per batch)
HPG = H // G     # heads per core = 4
FS = HPG * D     # feature slice per core = 256
P = 128
QB = 512         # query block (matmul moving width)
NQB = S // QB    # 4
NKC = S // P     # 16 k-chunks

_cache = {}


def _split_waits(nc, mybir, max_waits=1):
    """This walrus build encodes at most one sem-wait per instruction.
    Hoist extra waits onto NOPs inserted before the instruction in the same
    engine stream (same basic block => order preserved)."""
    uid = [0]
    for fn in nc.m.functions:
        for bb in fn.blocks:
            new = []
            changed = False
            for inst in bb.instructions:
                si = inst.sync_info
                if si is not None and len(si.on_wait) > max_waits:
                    waits = list(si.on_wait)
                    head, tail = waits[:-max_waits], waits[-max_waits:]
                    for k in range(0, len(head), max_waits):
                        nop = mybir.InstNoOp(name=f"WSPLIT-{uid[0]}", ins=[], outs=[])
                        uid[0] += 1
                        nop.engine = inst.engine
                        nop.sync_info = mybir.SyncInfo(
                            on_wait=head[k:k + max_waits], on_update=[])
                        new.append(nop)
                    inst.sync_info = mybir.SyncInfo(
                        on_wait=tail, on_update=list(si.on_update))
                    changed = True
                new.append(inst)
            if changed:
                bb.instructions = new


def _build(reps=1):
    key = ("nc", reps)
    if key in _cache:
        return _cache[key]
    import os
    no_warm = bool(os.environ.get("ABL_NOWARM"))
    bf16_qk = bool(os.environ.get("ABL_BF16QK"))
    split_s = bool(os.environ.get("ABL_SPLIT_S"))
    abl_exp = bool(os.environ.get("ABL_EXP"))      # timing-only: stub exp width
    abl_dma = bool(os.environ.get("ABL_DMA"))      # timing-only: stub x DMA
    abl_noout = bool(os.environ.get("ABL_NOOUT"))  # timing-only: stub out DMA

    import concourse.bass as bass
    import concourse.mybir as mybir
    import concourse.tile as tile

    F32 = mybir.dt.float32
    F32R = mybir.dt.float32r
    BF16 = mybir.dt.bfloat16
    EXP = mybir.ActivationFunctionType.Exp

    nc = bass.Bass("TRN2", target_bir_lowering=False, debug=False)

    xt_d = nc.dram_tensor("xt", [E, S], F32R, kind="ExternalInput")
    wq_d = nc.dram_tensor("wqt", [P, E // P, FS], F32R, kind="ExternalInput")
    wk_d = nc.dram_tensor("wkt", [P, E // P, FS], F32R, kind="ExternalInput")
    wv_d = nc.dram_tensor("wvt", [P, E // P, FS], F32R, kind="ExternalInput")
    wo_d = nc.dram_tensor("wot", [P, FS // P, E], F32R, kind="ExternalInput")
    bq_d = nc.dram_tensor("bq", [P, 2], F32, kind="ExternalInput")
    bk_d = nc.dram_tensor("bk", [P, 2], F32, kind="ExternalInput")
    bv_d = nc.dram_tensor("bvb", [P, FS], F32, kind="ExternalInput")   # pre-broadcast
    mask_d = nc.dram_tensor("mask", [P, P], F32R, kind="ExternalInput")  # tri: 1 if j>=k
    out_d = nc.dram_tensor("outt", [E, S], F32, kind="ExternalOutput")

    EC = E // P  # 8 contraction chunks for projections

    with tile.TileContext(nc) as tc, \
         nc.allow_low_precision(reason="fp32r/bf16 rounding for PE operands is intended"), \
         tc.tile_pool(name="big", bufs=1) as big, \
         tc.tile_pool(name="xr", bufs=3) as xrp, \
         tc.tile_pool(name="small", bufs=1) as small, \
         tc.tile_pool(name="pp", bufs=2, space="PSUM") as psp, \
         tc.tile_pool(name="psc", bufs=2, space="PSUM") as psc, \
         tc.tile_pool(name="pav", bufs=1, space="PSUM") as pav, \
         tc.tile_pool(name="pt", bufs=4) as ptp, \
         tc.tile_pool(name="rc", bufs=2) as rcp, \
         tc.tile_pool(name="ot", bufs=3) as otp:

        # ---- tiles that persist across reps (constant scratch) ----
        warm_f = small.tile([P, QB], F32, tag="warmf")
        warm_z = small.tile([P, QB], F32R, tag="warmz")
        tiny = small.tile([P, 2], BF16, tag="tiny")
        # per k-chunk, per head: [v (cols 0:D) | ones (cols D:2D)]; the v
        # columns are rewritten every rep, the ones columns are constant.
        vpad = [big.tile([P, HPG, 2 * D], BF16, name=f"vp{c}", tag=f"vp{c}")
                for c in range(NKC)]
        QKDT = BF16 if bf16_qk else F32R
        qT = [big.tile([P, S], QKDT, name=f"qT{f}", tag=f"qT{f}") for f in range(2)]
        kT = [big.tile([P, S], QKDT, name=f"kT{f}", tag=f"kT{f}") for f in range(2)]
        attnT = [big.tile([P, S], F32R, name=f"aT{f}", tag=f"aT{f}") for f in range(2)]

        nc.vector.memset(warm_f[:], 0.5)
        nc.vector.tensor_copy(warm_z[:], warm_f[:])
        for c in range(NKC):
            nc.vector.memset(vpad[c][:, :, D:2 * D], 1.0)
        # exp table preload (walrus inserts the table DMA before this)
        nc.scalar.activation(tiny[:], warm_f[:, 0:2], EXP)

        for _rep in range(reps):
            R = f"r{_rep}"
            # ---- per-rep input tiles: same tag -> same SBUF slot; the
            #      re-DMA only WAR-depends on the previous rep's readers,
            #      so the next rep's input stream overlaps this rep's
            #      ACT-bound attention tail. ----
            wq_t = small.tile([P, EC, FS], F32R, tag="wq", name=f"wq{R}")
            wk_t = small.tile([P, EC, FS], F32R, tag="wk", name=f"wk{R}")
            wv_t = small.tile([P, EC, FS], F32R, tag="wv", name=f"wv{R}")
            wo_t = small.tile([P, 2, E], F32R, tag="wo", name=f"wo{R}")
            bq_t = small.tile([P, 2], F32, tag="bq", name=f"bq{R}")
            bk_t = small.tile([P, 2], F32, tag="bk", name=f"bk{R}")
            bv_t = small.tile([P, FS], F32, tag="bv", name=f"bv{R}")
            mask_t = small.tile([P, P], F32R, tag="mask", name=f"mask{R}")
            maskb = small.tile([P, P], BF16, tag="maskb", name=f"maskb{R}")

            # x streams through a 3-slot ring of sequence blocks
            xr = [xrp.tile([P, EC, QB], F32R, tag="xr", name=f"xr{sc}{R}")
                  for sc in range(NQB)]

            nc.sync.dma_start(bq_t[:], bq_d[:])
            nc.sync.dma_start(bk_t[:], bk_d[:])
            nc.sync.dma_start(wq_t[:], wq_d[:])
            nc.gpsimd.dma_start(mask_t[:], mask_d[:])
            nc.gpsimd.dma_start(bv_t[:], bv_d[:])
            for sc in range(NQB):
                for c in range(EC):
                    eng = nc.sync if c % 2 == 0 else nc.gpsimd
                    _w = 1 if abl_dma else QB
                    eng.dma_start(
                        xr[sc][:, c, 0:_w],
                        xt_d[bass.ts(c, P), sc * QB:sc * QB + _w])
                if sc == 0:
                    nc.gpsimd.dma_start(wk_t[:], wk_d[:])
                    nc.sync.dma_start(wv_t[:], wv_d[:])
                    nc.gpsimd.dma_start(wo_t[:], wo_d[:])

            nc.vector.tensor_copy(maskb[:], mask_t[:])

            # ---- PE warmup on first rep: open the HAM clock gate during
            #      the cold-start DMA ramp ----
            if not no_warm and _rep == 0:
                wps = psp.tile([P, QB], F32, tag="pp", name=f"wps{R}")
                for wi in range(24):
                    nc.tensor.matmul(wps[:], warm_z[:, 0:P], warm_z[:],
                                     start=(wi == 0), stop=(wi == 23))

            bvv = bv_t.rearrange("p (h d) -> p h d", h=HPG)

            def out_proj(qb):
                for m in range(EC):
                    po = psp.tile([P, QB], F32, tag="pp", name=f"po{qb}{m}{R}")
                    nc.tensor.matmul(po[:], wo_t[:, 0, bass.ts(m, P)],
                                     attnT[0][:, bass.ts(qb, QB)],
                                     start=True, stop=False)
                    nc.tensor.matmul(po[:], wo_t[:, 1, bass.ts(m, P)],
                                     attnT[1][:, bass.ts(qb, QB)],
                                     start=False, stop=True)
                    ot = otp.tile([P, QB], F32, tag="ot")
                    nc.vector.tensor_copy(ot[:], po[:])
                    eng = nc.sync if m % 2 == 0 else nc.gpsimd
                    _w = 1 if abl_noout else QB
                    eng.dma_start(
                        out_d[bass.ts(m, P), qb * QB:qb * QB + _w], ot[:, 0:_w])

            for sc in range(NQB):
                # ---- projections for sequence block sc ----
                for fc in range(2):
                    for dst, w, bias in ((qT, wq_t, bq_t), (kT, wk_t, bk_t)):
                        ps = psp.tile([P, QB], F32, tag="pp")
                        for ec in range(EC):
                            nc.tensor.matmul(
                                ps[:], w[:, ec, bass.ts(fc, P)],
                                xr[sc][:, ec, :],
                                start=(ec == 0), stop=(ec == EC - 1))
                        nc.vector.tensor_add(
                            dst[fc][:, bass.ts(sc, QB)], ps[:],
                            bias[:, fc:fc + 1].to_broadcast((P, QB)))
                for lv in range(4):
                    sv = 4 * sc + lv
                    ps = psp.tile([P, FS], F32, tag="pp")
                    for ec in range(EC):
                        nc.tensor.matmul(
                            ps[:], xr[sc][:, ec, bass.ts(lv, P)], wv_t[:, ec, :],
                            start=(ec == 0), stop=(ec == EC - 1))
                    psv = ps.rearrange("p (h d) -> p h d", h=HPG)
                    nc.vector.tensor_add(vpad[sv][:, :, 0:D], psv[:], bvv[:])

                # ---- output projection of the previous q-block: fills the
                #      PE while this block's projections' psum drains and the
                #      previous softmax tail completes ----
                if sc > 0:
                    out_proj(sc - 1)

                # ---- attention for q-block sc ----
                q0 = sc * QB
                nch = 4 * (sc + 1)
                for hp in range(2):
                    fc = hp
                    heads = (2 * hp, 2 * hp + 1)
                    av = [pav.tile([P, QB], F32, name=f"av{hp}{i}{R}",
                                   tag=f"av{i}") for i in range(2)]
                    pend = []

                    def issue_av(entry, stop):
                        pt, d0, c0 = entry
                        for i, h in enumerate(heads):
                            nc.tensor.matmul(
                                av[i][:, d0:QB], vpad[c0][:, h, :],
                                pt[:, i, d0:QB],
                                start=(c0 == 0), stop=stop)

                    for c in range(nch):
                        delta = max(0, c * P - q0)
                        # f32r matmuls need moving dim >=256 for full rate
                        dmm = delta if bf16_qk else min(delta, 2 * P)
                        sp2 = psc.tile([P, 2, QB], F32, tag="sc")
                        for i in range(2):
                            ro = i * D
                            nc.tensor.matmul(
                                sp2[:, i, dmm:QB],
                                kT[fc][ro:ro + D, bass.ts(c, P)],
                                qT[fc][ro:ro + D, q0 + dmm:q0 + QB],
                                start=True, stop=True)
                            if split_s and i == 0 and len(pend) == 2:
                                issue_av(pend.pop(0), stop=False)
                        if len(pend) == 2:
                            issue_av(pend.pop(0), stop=False)
                        pt = ptp.tile([P, 2, QB], BF16, tag="pT")
                        _w = delta + 1 if abl_exp else QB
                        nc.scalar.activation(
                            pt[:, :, delta:_w], sp2[:, :, delta:_w], EXP,
                            scale=0.125)
                        if c * P >= q0:
                            nc.vector.tensor_mul(
                                pt[:, :, delta:delta + P],
                                pt[:, :, delta:delta + P],
                                maskb[:, None, :].to_broadcast((P, 2, P)))
                        pend.append((pt, delta, c))
                    while pend:
                        issue_av(pend.pop(0), stop=(not pend))
                    for i, h in enumerate(heads):
                        ro = i * D
                        rc = rcp.tile([P, QB], BF16, tag="rc")
                        nc.vector.reciprocal(rc[D:2 * D, :], av[i][D:2 * D, :])
                        nc.vector.tensor_mul(
                            attnT[fc][ro:ro + D, q0:q0 + QB],
                            av[i][0:D, :], rc[D:2 * D, :])

            out_proj(NQB - 1)

    _split_waits(nc, mybir)
    _cache[key] = nc
    return nc


def _ilv(w):
    """(C*128, N) -> (128, C, N): partition-major interleave for plain DMA."""
    c = w.shape[0] // P
    return np.ascontiguousarray(w.reshape(c, P, w.shape[1]).transpose(1, 0, 2))


def _in_maps(x, Wq, bq, Wk, bk, Wv, bv, Wo, bo):
    f32 = np.float32
    xT = [np.ascontiguousarray(x[b].T, dtype=f32) for b in range(B)]
    WqT = np.ascontiguousarray(Wq.T, dtype=f32)
    WkT = np.ascontiguousarray(Wk.T, dtype=f32)
    WvT = np.ascontiguousarray(Wv.T, dtype=f32)
    # out = attn @ Wo.T -> partial over feature slice: lhsT rows = local f
    tri = np.triu(np.ones((P, P), dtype=f32))  # [k, j] = 1 if j >= k
    maps = []
    for core in range(NCORES):
        b, g = divmod(core, G)
        fs = slice(g * FS, (g + 1) * FS)
        maps.append({
            "xt": xT[b],
            "wqt": _ilv(WqT[:, fs]),
            "wkt": _ilv(WkT[:, fs]),
            "wvt": _ilv(WvT[:, fs]),
            "wot": _ilv(Wo[:, fs].T),
            "bq": np.ascontiguousarray(bq[fs].reshape(2, P).T),
            "bk": np.ascontiguousarray(bk[fs].reshape(2, P).T),
            "bvb": np.broadcast_to(bv[fs], (P, FS)).copy(),
            "mask": tri,
        })
    return maps


def _runner(reps=1):
    """Compile once; return (exec_fn, put_fn).

    put_fn(maps) -> device args (inputs resident on the 8 cores).
    exec_fn(args) -> list of 8 per-core output dicts (numpy).
    """
    rkey = ("run", reps)
    if rkey in _cache:
        return _cache[rkey]

    import jax
    from jax.experimental.shard_map import shard_map
    from jax.sharding import Mesh, NamedSharding, PartitionSpec

    import concourse.mybir as mybir
    from concourse.bass2jax import (
        _bass_exec_p,
        install_neuronx_cc_hook,
        partition_id_tensor,
    )

    nc = _build(reps)
    install_neuronx_cc_hook()

    partition_name = nc.partition_id_tensor.name if nc.partition_id_tensor else None
    in_names, out_names, out_avals, zero_outs = [], [], [], []
    for alloc in nc.m.functions[0].allocations:
        if not isinstance(alloc, mybir.MemoryLocationSet):
            continue
        name = alloc.memorylocations[0].name
        if alloc.kind == "ExternalInput":
            if name != partition_name:
                in_names.append(name)
        elif alloc.kind == "ExternalOutput":
            shape = tuple(alloc.tensor_shape)
            dtype = mybir.dt.np(alloc.dtype)
            out_names.append(name)
            out_avals.append(jax.core.ShapedArray(shape, dtype))
            zero_outs.append(np.zeros(shape, dtype))
    n_params = len(in_names)
    all_in_names = list(in_names) + list(out_names)
    if partition_name is not None:
        all_in_names.append(partition_name)

    def _body(*args):
        operands = list(args)
        if partition_name is not None:
            operands.append(partition_id_tensor())
        outs = _bass_exec_p.bind(
            *operands,
            out_avals=tuple(out_avals),
            in_names=tuple(all_in_names),
            out_names=tuple(out_names),
            lowering_input_output_aliases=(),
            sim_require_finite=True,
            sim_require_nnan=True,
            nc=nc,
        )
        return tuple(outs)

    devices = jax.devices()[:NCORES]
    mesh = Mesh(np.asarray(devices), ("core",))
    n_ops = n_params + len(out_names)
    sharded = jax.jit(
        shard_map(
            _body, mesh=mesh,
            in_specs=(PartitionSpec("core"),) * n_ops,
            out_specs=(PartitionSpec("core"),) * len(out_names),
            check_rep=False,
        ),
        keep_unused=True,
    )
    shard = NamedSharding(mesh, PartitionSpec("core"))

    def put_fn(maps):
        concat = [
            np.concatenate([np.asarray(maps[c][n]) for c in range(NCORES)], axis=0)
            for n in in_names
        ] + [
            np.concatenate([z] * NCORES, axis=0) for z in zero_outs
        ]
        return [jax.device_put(a, shard) for a in concat]

    def exec_fn(args):
        out_arrs = sharded(*args)
        jax.block_until_ready(out_arrs)
        return [
            {
                n: np.asarray(out_arrs[i]).reshape(NCORES, *out_avals[i].shape)[c]
                for i, n in enumerate(out_names)
            }
            for c in range(NCORES)
        ]

    def time_fn(args):
        # device execution only: no output fetch to host
        out_arrs = sharded(*args)
        jax.block_until_ready(out_arrs)

    _cache[rkey] = (exec_fn, put_fn, time_fn)
    return _cache[rkey]


def _assemble(results, bo):
    out = np.empty((B, S, E), dtype=np.float32)
    for b in range(B):
        acc = results[b * G]["outt"].astype(np.float32)
        for g in range(1, G):
            acc = acc + results[b * G + g]["outt"]
        out[b] = acc.T + bo
    return out


def kernel(x, Wq, bq, Wk, bk, Wv, bv, Wo, bo):
    exec_fn, put_fn, _ = _runner()
    maps = _in_maps(x, Wq, bq, Wk, bk, Wv, bv, Wo, bo)
    args = put_fn(maps)
    if not _cache.get("warm"):
        # First execution after load can race device-side initialization;
        # run once and discard, then use the steady-state result.
        exec_fn(args)
        _cache["warm"] = True
    results = exec_fn(args)
    return _assemble(results, bo)


# revision 21
# speedup vs baseline: 1.0377x; 1.0377x over previous
"""Causal multi-head attention on 8 trn2 NeuronCores.

Sharding: core = (batch b in {0,1}) x (head-group g in {0..3}; 4 heads each).
QKV weights column-sharded, Wo row-sharded (Megatron TP); each core emits a
partial output for its batch; the host sums the 4 partials per batch and adds
the output bias (the unshard step for row-parallel sharding).

Structure: one fused loop over sequence blocks of 512.  Iteration sc projects
q/k/v for block sc, then runs flash-style causal attention for q-block sc
(whose k-extent is exactly what has been projected so far), with the output
projection of block sc-1 wedged between to fill the PE while the previous
block's softmax tail drains.  Input x streams in 512-column slices in
consumption order on two DMA queues so the first projection starts ~8us in.

On-chip layout is feature-major: xT (E,S), qT/kT (256,S).  v is kept in
natural (s,f) layout as [v | ones*64] bf16 so the AV matmul uses the full
128-wide array and produces the softmax denominators replicated across 64
psum partitions - the normalize is then reciprocal + one elementwise mul.
Scores are computed transposed (k,q); the two heads of a pair go to row
groups 0/64 of the PE array back-to-back (concurrent on hardware) and share
one batched exp.  Probabilities are bf16.  Softmax skips max-subtraction
(scores are O(5); exp is safe in fp32).
"""

import numpy as np

B, S, E, H, D = 2, 2048, 1024, 16, 64
NCORES = 8
G = 4            # head-groups (cores per batch)
HPG = H // G     # heads per core = 4
FS = HPG * D     # feature slice per core = 256
P = 128
QB = 512         # query block (matmul moving width)
NQB = S // QB    # 4
NKC = S // P     # 16 k-chunks

_cache = {}


def _split_waits(nc, mybir, max_waits=1):
    """This walrus build encodes at most one sem-wait per instruction.
    Hoist extra waits onto NOPs inserted before the instruction in the same
    engine stream (same basic block => order preserved)."""
    uid = [0]
    for fn in nc.m.functions:
        for bb in fn.blocks:
            new = []
            changed = False
            for inst in bb.instructions:
                si = inst.sync_info
                if si is not None and len(si.on_wait) > max_waits:
                    waits = list(si.on_wait)
                    head, tail = waits[:-max_waits], waits[-max_waits:]
                    for k in range(0, len(head), max_waits):
                        nop = mybir.InstNoOp(name=f"WSPLIT-{uid[0]}", ins=[], outs=[])
                        uid[0] += 1
                        nop.engine = inst.engine
                        nop.sync_info = mybir.SyncInfo(
                            on_wait=head[k:k + max_waits], on_update=[])
                        new.append(nop)
                    inst.sync_info = mybir.SyncInfo(
                        on_wait=tail, on_update=list(si.on_update))
                    changed = True
                new.append(inst)
            if changed:
                bb.instructions = new


def _build(reps=1):
    key = ("nc", reps)
    if key in _cache:
        return _cache[key]
    import os
    no_warm = bool(os.environ.get("ABL_NOWARM"))
    bf16_qk = bool(os.environ.get("ABL_BF16QK"))
    split_s = bool(os.environ.get("ABL_SPLIT_S"))
    abl_exp = bool(os.environ.get("ABL_EXP"))      # timing-only: stub exp width
    abl_dma = bool(os.environ.get("ABL_DMA"))      # timing-only: stub x DMA
    abl_noout = bool(os.environ.get("ABL_NOOUT"))  # timing-only: stub out DMA
    abl_dve = bool(os.environ.get("ABL_DVE"))      # timing-only: stub DVE widths

    import concourse.bass as bass
    import concourse.mybir as mybir
    import concourse.tile as tile

    F32 = mybir.dt.float32
    F32R = mybir.dt.float32r
    BF16 = mybir.dt.bfloat16
    EXP = mybir.ActivationFunctionType.Exp

    nc = bass.Bass("TRN2", target_bir_lowering=False, debug=False)

    xt_d = nc.dram_tensor("xt", [E, S], F32R, kind="ExternalInput")
    wq_d = nc.dram_tensor("wqt", [P, E // P, FS], F32R, kind="ExternalInput")
    wk_d = nc.dram_tensor("wkt", [P, E // P, FS], F32R, kind="ExternalInput")
    wv_d = nc.dram_tensor("wvt", [P, E // P, FS], F32R, kind="ExternalInput")
    wo_d = nc.dram_tensor("wot", [P, FS // P, E], F32R, kind="ExternalInput")
    bq_d = nc.dram_tensor("bq", [P, 2], F32, kind="ExternalInput")
    bk_d = nc.dram_tensor("bk", [P, 2], F32, kind="ExternalInput")
    bv_d = nc.dram_tensor("bvb", [P, FS], F32, kind="ExternalInput")   # pre-broadcast
    mask_d = nc.dram_tensor("mask", [P, P], F32R, kind="ExternalInput")  # tri: 1 if j>=k
    out_d = nc.dram_tensor("outt", [E, S], F32, kind="ExternalOutput")

    EC = E // P  # 8 contraction chunks for projections

    with tile.TileContext(nc) as tc, \
         nc.allow_low_precision(reason="fp32r/bf16 rounding for PE operands is intended"), \
         tc.tile_pool(name="big", bufs=1) as big, \
         tc.tile_pool(name="xr", bufs=3) as xrp, \
         tc.tile_pool(name="small", bufs=1) as small, \
         tc.tile_pool(name="pp", bufs=2, space="PSUM") as psp, \
         tc.tile_pool(name="psc", bufs=2, space="PSUM") as psc, \
         tc.tile_pool(name="pav", bufs=1, space="PSUM") as pav, \
         tc.tile_pool(name="pt", bufs=4) as ptp, \
         tc.tile_pool(name="rc", bufs=2) as rcp, \
         tc.tile_pool(name="ot", bufs=3) as otp:

        # ---- tiles that persist across reps (constant scratch) ----
        warm_f = small.tile([P, QB], F32, tag="warmf")
        warm_z = small.tile([P, QB], F32R, tag="warmz")
        tiny = small.tile([P, 2], BF16, tag="tiny")
        # per k-chunk, per head: [v (cols 0:D) | ones (cols D:2D)]; the v
        # columns are rewritten every rep, the ones columns are constant.
        vpad = [big.tile([P, HPG, 2 * D], BF16, name=f"vp{c}", tag=f"vp{c}")
                for c in range(NKC)]
        QKDT = BF16 if bf16_qk else F32R
        qT = [big.tile([P, S], QKDT, name=f"qT{f}", tag=f"qT{f}") for f in range(2)]
        kT = [big.tile([P, S], QKDT, name=f"kT{f}", tag=f"kT{f}") for f in range(2)]
        attnT = [big.tile([P, S], F32R, name=f"aT{f}", tag=f"aT{f}") for f in range(2)]

        nc.vector.memset(warm_f[:], 0.5)
        nc.vector.tensor_copy(warm_z[:], warm_f[:])
        for c in range(NKC):
            nc.vector.memset(vpad[c][:, :, D:2 * D], 1.0)
        # exp table preload (walrus inserts the table DMA before this)
        nc.scalar.activation(tiny[:], warm_f[:, 0:2], EXP)

        for _rep in range(reps):
            R = f"r{_rep}"
            # ---- per-rep input tiles: same tag -> same SBUF slot; the
            #      re-DMA only WAR-depends on the previous rep's readers,
            #      so the next rep's input stream overlaps this rep's
            #      ACT-bound attention tail. ----
            wq_t = small.tile([P, EC, FS], F32R, tag="wq", name=f"wq{R}")
            wk_t = small.tile([P, EC, FS], F32R, tag="wk", name=f"wk{R}")
            wv_t = small.tile([P, EC, FS], F32R, tag="wv", name=f"wv{R}")
            wo_t = small.tile([P, 2, E], F32R, tag="wo", name=f"wo{R}")
            bq_t = small.tile([P, 2], F32, tag="bq", name=f"bq{R}")
            bk_t = small.tile([P, 2], F32, tag="bk", name=f"bk{R}")
            bv_t = small.tile([P, FS], F32, tag="bv", name=f"bv{R}")
            mask_t = small.tile([P, P], F32R, tag="mask", name=f"mask{R}")
            maskb = small.tile([P, P], BF16, tag="maskb", name=f"maskb{R}")

            # x streams through a 3-slot ring of sequence blocks
            xr = [xrp.tile([P, EC, QB], F32R, tag="xr", name=f"xr{sc}{R}")
                  for sc in range(NQB)]

            nc.sync.dma_start(bq_t[:], bq_d[:])
            nc.sync.dma_start(bk_t[:], bk_d[:])
            nc.sync.dma_start(wq_t[:], wq_d[:])
            nc.gpsimd.dma_start(mask_t[:], mask_d[:])
            nc.gpsimd.dma_start(bv_t[:], bv_d[:])
            for sc in range(NQB):
                for c in range(EC):
                    eng = nc.sync if c % 2 == 0 else nc.gpsimd
                    _w = 1 if abl_dma else QB
                    eng.dma_start(
                        xr[sc][:, c, 0:_w],
                        xt_d[bass.ts(c, P), sc * QB:sc * QB + _w])
                if sc == 0:
                    nc.gpsimd.dma_start(wk_t[:], wk_d[:])
                    nc.sync.dma_start(wv_t[:], wv_d[:])
                    nc.gpsimd.dma_start(wo_t[:], wo_d[:])

            nc.vector.tensor_copy(maskb[:], mask_t[:])

            # ---- PE warmup on first rep: open the HAM clock gate during
            #      the cold-start DMA ramp ----
            if not no_warm and _rep == 0:
                wps = psp.tile([P, QB], F32, tag="pp", name=f"wps{R}")
                for wi in range(24):
                    nc.tensor.matmul(wps[:], warm_z[:, 0:P], warm_z[:],
                                     start=(wi == 0), stop=(wi == 23))

            bvv = bv_t.rearrange("p (h d) -> p h d", h=HPG)

            def out_proj(qb):
                for m in range(EC):
                    po = psp.tile([P, QB], F32, tag="pp", name=f"po{qb}{m}{R}")
                    nc.tensor.matmul(po[:], wo_t[:, 0, bass.ts(m, P)],
                                     attnT[0][:, bass.ts(qb, QB)],
                                     start=True, stop=False)
                    nc.tensor.matmul(po[:], wo_t[:, 1, bass.ts(m, P)],
                                     attnT[1][:, bass.ts(qb, QB)],
                                     start=False, stop=True)
                    ot = otp.tile([P, QB], F32, tag="ot")
                    _wc = 1 if abl_dve else QB
                    nc.vector.tensor_copy(ot[:, 0:_wc], po[:, 0:_wc])
                    eng = nc.sync if m % 2 == 0 else nc.gpsimd
                    _w = 1 if abl_noout else QB
                    eng.dma_start(
                        out_d[bass.ts(m, P), qb * QB:qb * QB + _w], ot[:, 0:_w])

            for sc in range(NQB):
                # ---- projections for sequence block sc ----
                for fc in range(2):
                    for dst, w, bias in ((qT, wq_t, bq_t), (kT, wk_t, bk_t)):
                        ps = psp.tile([P, QB], F32, tag="pp")
                        for ec in range(EC):
                            nc.tensor.matmul(
                                ps[:], w[:, ec, bass.ts(fc, P)],
                                xr[sc][:, ec, :],
                                start=(ec == 0), stop=(ec == EC - 1))
                        nc.vector.tensor_add(
                            dst[fc][:, bass.ts(sc, QB)], ps[:],
                            bias[:, fc:fc + 1].to_broadcast((P, QB)))
                for lv in range(4):
                    sv = 4 * sc + lv
                    ps = psp.tile([P, FS], F32, tag="pp")
                    for ec in range(EC):
                        nc.tensor.matmul(
                            ps[:], xr[sc][:, ec, bass.ts(lv, P)], wv_t[:, ec, :],
                            start=(ec == 0), stop=(ec == EC - 1))
                    psv = ps.rearrange("p (h d) -> p h d", h=HPG)
                    nc.vector.tensor_add(vpad[sv][:, :, 0:D], psv[:], bvv[:])

                # ---- output projection of the previous q-block: fills the
                #      PE while this block's projections' psum drains and the
                #      previous softmax tail completes ----
                if sc > 0:
                    out_proj(sc - 1)

                # ---- attention for q-block sc ----
                q0 = sc * QB
                nch = 4 * (sc + 1)
                for hp in range(2):
                    fc = hp
                    heads = (2 * hp, 2 * hp + 1)
                    av = [pav.tile([P, QB], F32, name=f"av{hp}{i}{R}",
                                   tag=f"av{i}") for i in range(2)]
                    pend = []

                    def issue_av(entry, stop):
                        pt, d0, c0 = entry
                        for i, h in enumerate(heads):
                            nc.tensor.matmul(
                                av[i][:, d0:QB], vpad[c0][:, h, :],
                                pt[:, i, d0:QB],
                                start=(c0 == 0), stop=stop)

                    for c in range(nch):
                        delta = max(0, c * P - q0)
                        # f32r matmuls need moving dim >=256 for full rate
                        dmm = delta if bf16_qk else min(delta, 2 * P)
                        sp2 = psc.tile([P, 2, QB], F32, tag="sc")
                        for i in range(2):
                            ro = i * D
                            nc.tensor.matmul(
                                sp2[:, i, dmm:QB],
                                kT[fc][ro:ro + D, bass.ts(c, P)],
                                qT[fc][ro:ro + D, q0 + dmm:q0 + QB],
                                start=True, stop=True)
                            if split_s and i == 0 and len(pend) == 2:
                                issue_av(pend.pop(0), stop=False)
                        if len(pend) == 2:
                            issue_av(pend.pop(0), stop=False)
                        pt = ptp.tile([P, 2, QB], BF16, tag="pT")
                        _w = delta + 1 if abl_exp else QB
                        nc.scalar.activation(
                            pt[:, :, delta:_w], sp2[:, :, delta:_w], EXP,
                            scale=0.125)
                        if c * P >= q0:
                            nc.vector.tensor_mul(
                                pt[:, :, delta:delta + P],
                                pt[:, :, delta:delta + P],
                                maskb[:, None, :].to_broadcast((P, 2, P)))
                        pend.append((pt, delta, c))
                    while pend:
                        issue_av(pend.pop(0), stop=(not pend))
                    for i, h in enumerate(heads):
                        ro = i * D
                        rc = rcp.tile([P, QB], BF16, tag="rc")
                        _wn = 1 if abl_dve else QB
                        nc.vector.reciprocal(rc[D:2 * D, 0:_wn], av[i][D:2 * D, 0:_wn])
                        nc.vector.tensor_mul(
                            attnT[fc][ro:ro + D, q0:q0 + _wn],
                            av[i][0:D, 0:_wn], rc[D:2 * D, 0:_wn])

            out_proj(NQB - 1)

    _split_waits(nc, mybir)
    _cache[key] = nc
    return nc


def _ilv(w):
    """(C*128, N) -> (128, C, N): partition-major interleave for plain DMA."""
    c = w.shape[0] // P
    return np.ascontiguousarray(w.reshape(c, P, w.shape[1]).transpose(1, 0, 2))


def _in_maps(x, Wq, bq, Wk, bk, Wv, bv, Wo, bo):
    f32 = np.float32
    xT = [np.ascontiguousarray(x[b].T, dtype=f32) for b in range(B)]
    WqT = np.ascontiguousarray(Wq.T, dtype=f32)
    WkT = np.ascontiguousarray(Wk.T, dtype=f32)
    WvT = np.ascontiguousarray(Wv.T, dtype=f32)
    # out = attn @ Wo.T -> partial over feature slice: lhsT rows = local f
    tri = np.triu(np.ones((P, P), dtype=f32))  # [k, j] = 1 if j >= k
    maps = []
    for core in range(NCORES):
        b, g = divmod(core, G)
        fs = slice(g * FS, (g + 1) * FS)
        maps.append({
            "xt": xT[b],
            "wqt": _ilv(WqT[:, fs]),
            "wkt": _ilv(WkT[:, fs]),
            "wvt": _ilv(WvT[:, fs]),
            "wot": _ilv(Wo[:, fs].T),
            "bq": np.ascontiguousarray(bq[fs].reshape(2, P).T),
            "bk": np.ascontiguousarray(bk[fs].reshape(2, P).T),
            "bvb": np.broadcast_to(bv[fs], (P, FS)).copy(),
            "mask": tri,
        })
    return maps


def _runner(reps=1):
    """Compile once; return (exec_fn, put_fn).

    put_fn(maps) -> device args (inputs resident on the 8 cores).
    exec_fn(args) -> list of 8 per-core output dicts (numpy).
    """
    rkey = ("run", reps)
    if rkey in _cache:
        return _cache[rkey]

    import jax
    from jax.experimental.shard_map import shard_map
    from jax.sharding import Mesh, NamedSharding, PartitionSpec

    import concourse.mybir as mybir
    from concourse.bass2jax import (
        _bass_exec_p,
        install_neuronx_cc_hook,
        partition_id_tensor,
    )

    nc = _build(reps)
    install_neuronx_cc_hook()

    partition_name = nc.partition_id_tensor.name if nc.partition_id_tensor else None
    in_names, out_names, out_avals, zero_outs = [], [], [], []
    for alloc in nc.m.functions[0].allocations:
        if not isinstance(alloc, mybir.MemoryLocationSet):
            continue
        name = alloc.memorylocations[0].name
        if alloc.kind == "ExternalInput":
            if name != partition_name:
                in_names.append(name)
        elif alloc.kind == "ExternalOutput":
            shape = tuple(alloc.tensor_shape)
            dtype = mybir.dt.np(alloc.dtype)
            out_names.append(name)
            out_avals.append(jax.core.ShapedArray(shape, dtype))
            zero_outs.append(np.zeros(shape, dtype))
    n_params = len(in_names)
    all_in_names = list(in_names) + list(out_names)
    if partition_name is not None:
        all_in_names.append(partition_name)

    def _body(*args):
        operands = list(args)
        if partition_name is not None:
            operands.append(partition_id_tensor())
        outs = _bass_exec_p.bind(
            *operands,
            out_avals=tuple(out_avals),
            in_names=tuple(all_in_names),
            out_names=tuple(out_names),
            lowering_input_output_aliases=(),
            sim_require_finite=True,
            sim_require_nnan=True,
            nc=nc,
        )
        return tuple(outs)

    devices = jax.devices()[:NCORES]
    mesh = Mesh(np.asarray(devices), ("core",))
    n_ops = n_params + len(out_names)
    sharded = jax.jit(
        shard_map(
            _body, mesh=mesh,
            in_specs=(PartitionSpec("core"),) * n_ops,
            out_specs=(PartitionSpec("core"),) * len(out_names),
            check_rep=False,
        ),
        keep_unused=True,
    )
    shard = NamedSharding(mesh, PartitionSpec("core"))

    def put_fn(maps):
        concat = [
            np.concatenate([np.asarray(maps[c][n]) for c in range(NCORES)], axis=0)
            for n in in_names
        ] + [
            np.concatenate([z] * NCORES, axis=0) for z in zero_outs
        ]
        return [jax.device_put(a, shard) for a in concat]

    def exec_fn(args):
        out_arrs = sharded(*args)
        jax.block_until_ready(out_arrs)
        return [
            {
                n: np.asarray(out_arrs[i]).reshape(NCORES, *out_avals[i].shape)[c]
                for i, n in enumerate(out_names)
            }
            for c in range(NCORES)
        ]

    def time_fn(args):
        # device execution only: no output fetch to host
        out_arrs = sharded(*args)
        jax.block_until_ready(out_arrs)

    _cache[rkey] = (exec_fn, put_fn, time_fn)
    return _cache[rkey]


def _assemble(results, bo):
    out = np.empty((B, S, E), dtype=np.float32)
    for b in range(B):
        acc = results[b * G]["outt"].astype(np.float32)
        for g in range(1, G):
            acc = acc + results[b * G + g]["outt"]
        out[b] = acc.T + bo
    return out


def kernel(x, Wq, bq, Wk, bk, Wv, bv, Wo, bo):
    exec_fn, put_fn, _ = _runner()
    maps = _in_maps(x, Wq, bq, Wk, bk, Wv, bv, Wo, bo)
    args = put_fn(maps)
    if not _cache.get("warm"):
        # First execution after load can race device-side initialization;
        # run once and discard, then use the steady-state result.
        exec_fn(args)
        _cache["warm"] = True
    results = exec_fn(args)
    return _assemble(results, bo)
